# revision 3
# baseline (speedup 1.0000x reference)
"""Dense image warp (tfa.dense_image_warp semantics) on 8 Trainium2 NeuronCores.

The axon tunnel to the devices moves ~35 MB/s H2D / ~20 MB/s D2H, so the
kernel is wire-bound: the design minimizes bytes on the wire.

Sharding: pure data parallel, 8 shards = (batch 0..3) x (row-half 0..1);
each core warps 360 rows x 1280 cols x 16 ch of one frame.

Wire format (per core):
  - image shard as int8 (quantized to +-126.49/s, exact-int f32 math on
    device afterwards), ~7.9 MB instead of a 125 MB f32 quad table;
  - bilinear indices as compact int16 [16, n] (the 8x gpsimd-core
    replication the gather needs is done on device);
  - lerp weights as fp16 (upcast on device);
  - output as uint8 (result is a convex combination of int8 values, so it
    fits; bias +128 then RNE cast on device, dequantized on host).

Device algorithm per core:
  1. quad-table build: int8 image tiles -> SBUF, upcast to f32, then 4
     strided DMA writes per tile assemble the 256 B-per-position table
     [img[r,j], img[r,j+1], img[r+1,j], img[r+1,j+1]] in Internal DRAM
     (dma_gather requires elements and strides in 256 B units - pixel
     granularity indexing of the raw image is impossible);
  2. the four bilinear neighbours of every output pixel are fetched with
     ONE int16 dma_gather from that table (450 chunks of 1024 indices;
     chunk windows are band-rebased so indices fit int16);
  3. the two bilinear lerps run on the Vector engine with per-pixel fp16
     weights broadcast over the 16 channels; the +128 biased result is
     RNE-cast to uint8 and streamed out with large DMAs.

Host prepares (exact f32 math mirroring the reference): quantized image
shards, band-local indices in dma_gather's wrapped-16 layout, fp16 weights
in the gather's output layout; afterwards dequantizes the uint8 result.
"""

import os
import sys
import time

import numpy as np

import concourse.bass as bass
import concourse.mybir as mybir
from concourse import bacc
from concourse.tile import TileContext
from concourse.bass_utils import run_bass_kernel_spmd

# problem geometry (fixed per spec)
N, H, W, C = 4, 720, 1280, 16
HALF = H // 2                      # output rows per core
P = 128
K = 1024                           # indices per dma_gather (ring-safe)
SLOTS = K // P                     # 8
NCHUNK = (HALF * W) // K           # 450
G = 18                             # chunks per super-group
NSG = NCHUNK // G                  # 25
QROWS = H - 1                      # 719 quad rows
QCOLS = W - 1                      # 1279 quad cols
CW = 4 * C                         # 64 f32 = 256 B per quad position
IMG_Q = 126.49                     # int8 quant range; keeps hd+128 in [2,254]

_WRAP16 = np.arange(K // 16)[None, :] * 16 + np.arange(16)[:, None]
_REL = (np.arange(NCHUNK) * K) // W   # first output row (in-half) per chunk

_VERBOSE = bool(os.environ.get("KERNEL_VERBOSE"))


def _t(label, t0):
    if _VERBOSE:
        print(f"[kernel] {label}: {time.perf_counter() - t0:.3f}s",
              file=sys.stderr, flush=True)
    return time.perf_counter()


_PROGRAM_CACHE = {}


def _build_program(margin):
    key = margin
    if key in _PROGRAM_CACHE:
        return _PROGRAM_CACHE[key]
    win_len = 2 * margin + (K // W) + 4
    tq_rows = HALF + 2 * margin + (K // W) + 6
    rtiles = -(-(tq_rows + 1) // P)        # image-row tiles of 128
    rpad = rtiles * P

    nc = bacc.Bacc("TRN2", target_bir_lowering=False, debug=False, num_devices=8)
    img8 = nc.dram_tensor("img8", [rpad, W * C], mybir.dt.int8,
                          kind="ExternalInput")
    widx = nc.dram_tensor("widx", [16, NCHUNK * (K // 16)], mybir.dt.int16,
                          kind="ExternalInput")
    wab = nc.dram_tensor("wab", [P, NCHUNK * SLOTS * 2], mybir.dt.float16,
                         kind="ExternalInput")
    out = nc.dram_tensor("out", [P, NCHUNK * SLOTS * C], mybir.dt.uint8,
                         kind="ExternalOutput")
    imgq = nc.dram_tensor("imgq", [tq_rows * QCOLS, CW], mybir.dt.float32,
                          kind="Internal")

    with TileContext(nc) as tc:
        # ---- phase 1: build the quad table in device DRAM ----
        qv = imgq.ap().rearrange("(r j) (s c) -> r j s c", j=QCOLS, s=4)
        with tc.tile_pool(name="bld", bufs=1) as bld:
            for t in range(rtiles):
                r0 = t * P
                t8 = bld.tile([P, W, C], mybir.dt.int8, tag="t8")
                nc.sync.dma_start(
                    out=t8[:].rearrange("p a b -> p (a b)"),
                    in_=img8[r0:r0 + P, :])
                tf = bld.tile([P, W, C], mybir.dt.float32, tag="tf")
                nc.scalar.copy(out=tf[:], in_=t8[:])

                # DMA dst dims [rows, QCOLS] merge into one descriptor dim
                # (row stride == QCOLS * 256 B), whose num_elem field is
                # 16-bit: keep rows-per-DMA * QCOLS <= 65535.
                RCH = 50
                q1 = min(r0 + P, tq_rows)          # slabs 0/1: rows r0..q1
                qa = max(r0 - 1, 0)                # slabs 2/3: rows qa..qb
                qb = min(r0 + P - 1, tq_rows)
                for ra in range(r0, q1, RCH):
                    rb = min(ra + RCH, q1)
                    nc.sync.dma_start(out=qv[ra:rb, :, 0],
                                      in_=tf[ra - r0:rb - r0, 0:QCOLS])
                    nc.sync.dma_start(out=qv[ra:rb, :, 1],
                                      in_=tf[ra - r0:rb - r0, 1:W])
                for ra in range(qa, qb, RCH):
                    rb = min(ra + RCH, qb)
                    nc.sync.dma_start(out=qv[ra:rb, :, 2],
                                      in_=tf[ra - r0 + 1:rb - r0 + 1, 0:QCOLS])
                    nc.sync.dma_start(out=qv[ra:rb, :, 3],
                                      in_=tf[ra - r0 + 1:rb - r0 + 1, 1:W])

        # ---- phase 2: gather + bilinear lerp ----
        with (
            tc.tile_pool(name="idx", bufs=2) as idx_pool,
            tc.tile_pool(name="w", bufs=2) as w_pool,
            tc.tile_pool(name="g", bufs=2) as g_pool,
            tc.tile_pool(name="t", bufs=2) as t_pool,
        ):
            iw = K // 16
            for sg in range(NSG):
                idx_t = idx_pool.tile([P, G * iw], mybir.dt.int16, tag="idx")
                for g in range(8):                 # replicate for 8 gpsimd cores
                    nc.sync.dma_start(
                        out=idx_t[g * 16:(g + 1) * 16, :],
                        in_=widx[:, sg * G * iw:(sg + 1) * G * iw])
                w16 = w_pool.tile([P, G * SLOTS, 2], mybir.dt.float16, tag="w16")
                nc.sync.dma_start(
                    out=w16[:].rearrange("p a b -> p (a b)"),
                    in_=wab[:, sg * G * SLOTS * 2:(sg + 1) * G * SLOTS * 2])
                w32 = w_pool.tile([P, G * SLOTS, 2], mybir.dt.float32, tag="w32")
                nc.scalar.copy(out=w32[:], in_=w16[:])

                g_t = g_pool.tile([P, G, SLOTS, CW], mybir.dt.float32, tag="g")
                for j in range(G):
                    off = _REL[sg * G + j] * QCOLS
                    nc.gpsimd.dma_gather(
                        out_ap=g_t[:, j],
                        in_ap=imgq[off:off + win_len * QCOLS, :],
                        idxs_ap=idx_t[:, j * iw:(j + 1) * iw],
                        num_idxs=K, num_idxs_reg=K, elem_size=CW,
                    )

                npx = G * SLOTS
                gv = g_t[:].rearrange("p a b c -> p (a b) c")     # [P, npx, 64]
                ax = w32[:, :, 0:1]
                ay = w32[:, :, 1:2]

                dif = t_pool.tile([P, npx, 32], mybir.dt.float32, tag="dif")
                nc.vector.tensor_tensor(out=dif[:], in0=gv[:, :, 32:64],
                                        in1=gv[:, :, 0:32],
                                        op=mybir.AluOpType.subtract)
                ay_b, dif_b = bass.broadcast_tensor_aps(ay, dif[:])
                nc.vector.tensor_tensor(out=dif[:], in0=dif_b, in1=ay_b,
                                        op=mybir.AluOpType.mult)
                nc.vector.tensor_tensor(out=dif[:], in0=dif[:],
                                        in1=gv[:, :, 0:32],
                                        op=mybir.AluOpType.add)
                hd = t_pool.tile([P, npx, 16], mybir.dt.float32, tag="hd")
                nc.vector.tensor_tensor(out=hd[:], in0=dif[:, :, 16:32],
                                        in1=dif[:, :, 0:16],
                                        op=mybir.AluOpType.subtract)
                ax_b, hd_b = bass.broadcast_tensor_aps(ax, hd[:])
                nc.vector.tensor_tensor(out=hd[:], in0=hd_b, in1=ax_b,
                                        op=mybir.AluOpType.mult)
                nc.vector.tensor_tensor(out=hd[:], in0=hd[:],
                                        in1=dif[:, :, 0:16],
                                        op=mybir.AluOpType.add)

                u8 = t_pool.tile([P, npx, C], mybir.dt.uint8, tag="u8")
                nc.scalar.activation(out=u8[:], in_=hd[:],
                                     func=mybir.ActivationFunctionType.Copy,
                                     bias=128.0)
                nc.sync.dma_start(
                    out=out[:, sg * G * SLOTS * C:(sg + 1) * G * SLOTS * C],
                    in_=u8[:].rearrange("p a b -> p (a b)"))
    nc.compile()
    _PROGRAM_CACHE[key] = (nc, win_len, tq_rows, rpad)
    return _PROGRAM_CACHE[key]


def kernel(image, flow):
    t0 = time.perf_counter()
    image = np.asarray(image, dtype=np.float32)
    flow = np.asarray(flow, dtype=np.float32)
    f32 = np.float32

    s = float(np.abs(image).max())
    margin = int(np.ceil(np.abs(flow).max())) + 2
    nc, win_len, tq_rows, rpad = _build_program(margin)
    t0 = _t("program", t0)

    # int8 quantization; values are exact small ints in f32 on device
    img8_full = np.rint(image * f32(IMG_Q / s)).astype(np.int8)
    img8_full = img8_full.reshape(N, H, W * C)
    t0 = _t("quantize", t0)

    gi = np.arange(H, dtype=f32)[:, None]
    gj = np.arange(W, dtype=f32)[None, :]

    in_maps = []
    for core in range(8):
        b, h = core // 2, core % 2
        rows = slice(h * HALF, (h + 1) * HALF)

        # padded per-core int8 shard: shard row t <-> absolute row t + A
        A = h * HALF - margin
        shard = np.zeros((rpad, W * C), dtype=np.int8)
        lo = max(0, A)
        hi = min(H, A + tq_rows + 1)
        shard[lo - A:hi - A] = img8_full[b, lo:hi]

        f0 = flow[b, rows, :, 0]
        f1 = flow[b, rows, :, 1]
        qy = gi[rows] - f0
        qx = gj - f1
        fy = np.clip(np.floor(qy), 0.0, f32(H - 2)).astype(f32)
        fx = np.clip(np.floor(qx), 0.0, f32(W - 2)).astype(f32)
        ayw = np.clip(qy - fy, 0.0, 1.0).astype(f32)
        axw = np.clip(qx - fx, 0.0, 1.0).astype(f32)
        iy = fy.astype(np.int32).reshape(NCHUNK, K)
        ix = fx.astype(np.int32).reshape(NCHUNK, K)

        lrow = iy - A - _REL[:, None]
        loc = lrow * QCOLS + ix
        assert loc.min() >= 0 and loc.max() < win_len * QCOLS, (
            f"core {core} index out of window")
        widx16 = loc.astype(np.int16)
        widx = np.ascontiguousarray(
            widx16[:, _WRAP16].transpose(1, 0, 2).reshape(16, NCHUNK * (K // 16)))

        wflat = np.stack([axw.reshape(-1), ayw.reshape(-1)], axis=-1)
        wq = wflat.reshape(NCHUNK, SLOTS, P, 2).transpose(2, 0, 1, 3)
        wab = np.ascontiguousarray(
            wq.reshape(P, NCHUNK * SLOTS * 2)).astype(np.float16)

        in_maps.append({"img8": shard, "widx": widx, "wab": wab})
    t0 = _t("host prep", t0)

    res = run_bass_kernel_spmd(nc, in_maps, core_ids=list(range(8)))
    t0 = _t("device run", t0)

    full = np.empty((N, H, W, C), dtype=np.float32)
    dq = f32(s / IMG_Q)
    for core in range(8):
        b, h = core // 2, core % 2
        o = res.results[core]["out"].reshape(P, NCHUNK, SLOTS, C)
        o = o.transpose(1, 2, 0, 3).reshape(HALF, W, C)
        dst = full[b, h * HALF:(h + 1) * HALF]
        np.subtract(o.astype(f32), f32(128.0), out=dst)
        dst *= dq
    _t("postprocess", t0)
    return full


# revision 9
# speedup vs baseline: 1.1988x; 1.1988x over previous
"""Dense image warp (tfa.dense_image_warp semantics) on 8 Trainium2 NeuronCores.

The axon tunnel to the devices moves ~35 MB/s H2D / ~20 MB/s D2H, so the
kernel is wire-bound: the design minimizes bytes on the wire.

Sharding: pure data parallel, 8 shards = (batch 0..3) x (row-half 0..1);
each core warps 360 rows x 1280 cols x 16 ch of one frame.

Wire format (per core):
  - image shard as int8 (quantized to +-126.49/s, exact-int f32 math on
    device afterwards), ~7.9 MB instead of a 125 MB f32 quad table;
  - bilinear indices as compact int16 [16, n] (the 8x gpsimd-core
    replication the gather needs is done on device);
  - lerp weights as fp16 (upcast on device);
  - output as uint8 (result is a convex combination of int8 values, so it
    fits; bias +128 then RNE cast on device, dequantized on host).

Device algorithm per core:
  1. quad-table build: int8 image tiles -> SBUF, upcast to f32, then 4
     strided DMA writes per tile assemble the 256 B-per-position table
     [img[r,j], img[r,j+1], img[r+1,j], img[r+1,j+1]] in Internal DRAM
     (dma_gather requires elements and strides in 256 B units - pixel
     granularity indexing of the raw image is impossible);
  2. the four bilinear neighbours of every output pixel are fetched with
     ONE int16 dma_gather from that table (450 chunks of 1024 indices;
     chunk windows are band-rebased so indices fit int16);
  3. the two bilinear lerps run on the Vector engine with per-pixel fp16
     weights broadcast over the 16 channels; the +128 biased result is
     RNE-cast to uint8 and streamed out with large DMAs.

Host prepares (exact f32 math mirroring the reference): quantized image
shards, band-local indices in dma_gather's wrapped-16 layout, fp16 weights
in the gather's output layout; afterwards dequantizes the uint8 result.
"""

import os
import sys
import time

import numpy as np

import concourse.bass as bass
import concourse.mybir as mybir
from concourse import bacc
from concourse.tile import TileContext
from concourse.bass_utils import run_bass_kernel_spmd

# problem geometry (fixed per spec)
N, H, W, C = 4, 720, 1280, 16
HALF = H // 2                      # output rows per core
P = 128
K = 1024                           # indices per dma_gather (ring-safe)
SLOTS = K // P                     # 8
NCHUNK = (HALF * W) // K           # 450
G = 18                             # chunks per super-group
NSG = NCHUNK // G                  # 25
QROWS = H - 1                      # 719 quad rows
QCOLS = W - 1                      # 1279 quad cols
CW = 4 * C                         # 64 f32 = 256 B per quad position
IMG_Q = 126.49                     # int8 quant range; keeps hd+128 in [2,254]

_WRAP16 = np.arange(K // 16)[None, :] * 16 + np.arange(16)[:, None]
_REL = (np.arange(NCHUNK) * K) // W   # first output row (in-half) per chunk

_VERBOSE = bool(os.environ.get("KERNEL_VERBOSE"))


def _t(label, t0):
    if _VERBOSE:
        print(f"[kernel] {label}: {time.perf_counter() - t0:.3f}s",
              file=sys.stderr, flush=True)
    return time.perf_counter()


_PROGRAM_CACHE = {}


def _build_program(margin):
    key = margin
    if key in _PROGRAM_CACHE:
        return _PROGRAM_CACHE[key]
    win_len = 2 * margin + (K // W) + 4
    tq_rows = HALF + 2 * margin + (K // W) + 6
    rtiles = -(-(tq_rows + 1) // P)        # image-row tiles of 128
    rpad = rtiles * P

    nc = bacc.Bacc("TRN2", target_bir_lowering=False, debug=False, num_devices=8)
    img8 = nc.dram_tensor("img8", [rpad, W * C], mybir.dt.int8,
                          kind="ExternalInput")
    widx = nc.dram_tensor("widx", [16, NCHUNK * (K // 16)], mybir.dt.int16,
                          kind="ExternalInput")
    wab = nc.dram_tensor("wab", [P, NCHUNK * SLOTS * 2], mybir.dt.float16,
                         kind="ExternalInput")
    # pixel-major: row (ck, s), inner (p, c) -> linear pixel ck*1024+s*128+p,
    # so the host needs no permute, just dequantize.
    out = nc.dram_tensor("out", [NCHUNK * SLOTS, P * C], mybir.dt.uint8,
                         kind="ExternalOutput")
    imgq = nc.dram_tensor("imgq", [tq_rows * QCOLS, CW], mybir.dt.float32,
                          kind="Internal")

    with TileContext(nc) as tc:
        # ---- phase 1: build the quad table in device DRAM ----
        qv = imgq.ap().rearrange("(r j) (s c) -> r j s c", j=QCOLS, s=4)
        ov = out.ap().rearrange("r (p c) -> p r c", p=P)
        with tc.tile_pool(name="bld", bufs=1) as bld:
            for t in range(rtiles):
                r0 = t * P
                t8 = bld.tile([P, W, C], mybir.dt.int8, tag="t8")
                nc.sync.dma_start(
                    out=t8[:].rearrange("p a b -> p (a b)"),
                    in_=img8[r0:r0 + P, :])
                tf = bld.tile([P, W, C], mybir.dt.float32, tag="tf")
                nc.scalar.copy(out=tf[:], in_=t8[:])

                # DMA dst dims [rows, QCOLS] merge into one descriptor dim
                # (row stride == QCOLS * 256 B), whose num_elem field is
                # 16-bit: keep rows-per-DMA * QCOLS <= 65535.
                RCH = 50
                q1 = min(r0 + P, tq_rows)          # slabs 0/1: rows r0..q1
                qa = max(r0 - 1, 0)                # slabs 2/3: rows qa..qb
                qb = min(r0 + P - 1, tq_rows)
                for ra in range(r0, q1, RCH):
                    rb = min(ra + RCH, q1)
                    nc.sync.dma_start(out=qv[ra:rb, :, 0],
                                      in_=tf[ra - r0:rb - r0, 0:QCOLS])
                    nc.sync.dma_start(out=qv[ra:rb, :, 1],
                                      in_=tf[ra - r0:rb - r0, 1:W])
                for ra in range(qa, qb, RCH):
                    rb = min(ra + RCH, qb)
                    nc.sync.dma_start(out=qv[ra:rb, :, 2],
                                      in_=tf[ra - r0 + 1:rb - r0 + 1, 0:QCOLS])
                    nc.sync.dma_start(out=qv[ra:rb, :, 3],
                                      in_=tf[ra - r0 + 1:rb - r0 + 1, 1:W])

        # ---- phase 2: gather + bilinear lerp ----
        with (
            tc.tile_pool(name="idx", bufs=2) as idx_pool,
            tc.tile_pool(name="w", bufs=2) as w_pool,
            tc.tile_pool(name="g", bufs=2) as g_pool,
            tc.tile_pool(name="t", bufs=2) as t_pool,
        ):
            iw = K // 16
            for sg in range(NSG):
                idx_t = idx_pool.tile([P, G * iw], mybir.dt.int16, tag="idx")
                for g in range(8):                 # replicate for 8 gpsimd cores
                    nc.sync.dma_start(
                        out=idx_t[g * 16:(g + 1) * 16, :],
                        in_=widx[:, sg * G * iw:(sg + 1) * G * iw])
                w16 = w_pool.tile([P, G * SLOTS, 2], mybir.dt.float16, tag="w16")
                nc.sync.dma_start(
                    out=w16[:].rearrange("p a b -> p (a b)"),
                    in_=wab[:, sg * G * SLOTS * 2:(sg + 1) * G * SLOTS * 2])
                w32 = w_pool.tile([P, G * SLOTS, 2], mybir.dt.float32, tag="w32")
                nc.scalar.copy(out=w32[:], in_=w16[:])

                g_t = g_pool.tile([P, G, SLOTS, CW], mybir.dt.float32, tag="g")
                for j in range(G):
                    off = _REL[sg * G + j] * QCOLS
                    nc.gpsimd.dma_gather(
                        out_ap=g_t[:, j],
                        in_ap=imgq[off:off + win_len * QCOLS, :],
                        idxs_ap=idx_t[:, j * iw:(j + 1) * iw],
                        num_idxs=K, num_idxs_reg=K, elem_size=CW,
                    )

                npx = G * SLOTS
                gv = g_t[:].rearrange("p a b c -> p (a b) c")     # [P, npx, 64]
                ax = w32[:, :, 0:1]
                ay = w32[:, :, 1:2]

                dif = t_pool.tile([P, npx, 32], mybir.dt.float32, tag="dif")
                nc.vector.tensor_tensor(out=dif[:], in0=gv[:, :, 32:64],
                                        in1=gv[:, :, 0:32],
                                        op=mybir.AluOpType.subtract)
                ay_b, dif_b = bass.broadcast_tensor_aps(ay, dif[:])
                nc.vector.tensor_tensor(out=dif[:], in0=dif_b, in1=ay_b,
                                        op=mybir.AluOpType.mult)
                nc.vector.tensor_tensor(out=dif[:], in0=dif[:],
                                        in1=gv[:, :, 0:32],
                                        op=mybir.AluOpType.add)
                hd = t_pool.tile([P, npx, 16], mybir.dt.float32, tag="hd")
                nc.vector.tensor_tensor(out=hd[:], in0=dif[:, :, 16:32],
                                        in1=dif[:, :, 0:16],
                                        op=mybir.AluOpType.subtract)
                ax_b, hd_b = bass.broadcast_tensor_aps(ax, hd[:])
                nc.vector.tensor_tensor(out=hd[:], in0=hd_b, in1=ax_b,
                                        op=mybir.AluOpType.mult)
                nc.vector.tensor_tensor(out=hd[:], in0=hd[:],
                                        in1=dif[:, :, 0:16],
                                        op=mybir.AluOpType.add)

                u8 = t_pool.tile([P, npx, C], mybir.dt.uint8, tag="u8")
                nc.scalar.activation(out=u8[:], in_=hd[:],
                                     func=mybir.ActivationFunctionType.Copy,
                                     bias=128.0)
                nc.sync.dma_start(
                    out=ov[:, sg * G * SLOTS:(sg + 1) * G * SLOTS, :],
                    in_=u8[:])
    nc.compile()
    _PROGRAM_CACHE[key] = (nc, win_len, tq_rows, rpad)
    return _PROGRAM_CACHE[key]


_SCRATCH = {}


def kernel(image, flow):
    t0 = time.perf_counter()
    image = np.asarray(image, dtype=np.float32)
    flow = np.asarray(flow, dtype=np.float32)
    f32 = np.float32

    # max-abs without np.abs temporaries
    s = float(max(image.max(), -float(image.min())))
    margin = int(np.ceil(max(flow.max(), -float(flow.min())))) + 2
    nc, win_len, tq_rows, rpad = _build_program(margin)
    t0 = _t("program", t0)

    # int8 quantization; values are exact small ints in f32 on device.
    # Chunked + in-place into persistent scratch to avoid 236 MB temps.
    if not _SCRATCH:
        _SCRATCH["img8"] = np.empty((N, H, W * C), dtype=np.int8)
        _SCRATCH["tmp32"] = np.empty((H // 4, W * C), dtype=np.float32)
        _SCRATCH["shards"] = None
    img8_full = _SCRATCH["img8"]
    tmp32 = _SCRATCH["tmp32"]
    q = f32(IMG_Q / s)
    im2d = image.reshape(N * 4, H // 4, W * C)
    i82d = img8_full.reshape(N * 4, H // 4, W * C)
    for i in range(N * 4):
        np.multiply(im2d[i], q, out=tmp32)
        np.rint(tmp32, out=tmp32)
        np.copyto(i82d[i], tmp32, casting="unsafe")
    t0 = _t("quantize", t0)

    gi = np.arange(H, dtype=f32)[:, None]
    gj = np.arange(W, dtype=f32)[None, :]

    if _SCRATCH["shards"] is None or _SCRATCH["shards"][0].shape[0] != rpad:
        # pad rows stay zero across calls; only the valid span is rewritten
        _SCRATCH["shards"] = [np.zeros((rpad, W * C), dtype=np.int8)
                              for _ in range(8)]

    in_maps = []
    for core in range(8):
        b, h = core // 2, core % 2
        rows = slice(h * HALF, (h + 1) * HALF)

        # padded per-core int8 shard: shard row t <-> absolute row t + A
        A = h * HALF - margin
        shard = _SCRATCH["shards"][core]
        lo = max(0, A)
        hi = min(H, A + tq_rows + 1)
        shard[lo - A:hi - A] = img8_full[b, lo:hi]

        f0 = flow[b, rows, :, 0]
        f1 = flow[b, rows, :, 1]
        qy = gi[rows] - f0
        qx = gj - f1
        fy = np.clip(np.floor(qy), 0.0, f32(H - 2)).astype(f32)
        fx = np.clip(np.floor(qx), 0.0, f32(W - 2)).astype(f32)
        ayw = np.clip(qy - fy, 0.0, 1.0).astype(f32)
        axw = np.clip(qx - fx, 0.0, 1.0).astype(f32)
        iy = fy.astype(np.int32).reshape(NCHUNK, K)
        ix = fx.astype(np.int32).reshape(NCHUNK, K)

        lrow = iy - A - _REL[:, None]
        loc = lrow * QCOLS + ix
        assert loc.min() >= 0 and loc.max() < win_len * QCOLS, (
            f"core {core} index out of window")
        widx16 = loc.astype(np.int16)
        widx = np.ascontiguousarray(
            widx16[:, _WRAP16].transpose(1, 0, 2).reshape(16, NCHUNK * (K // 16)))

        wflat = np.stack([axw.reshape(-1), ayw.reshape(-1)], axis=-1)
        wq = wflat.reshape(NCHUNK, SLOTS, P, 2).transpose(2, 0, 1, 3)
        wab = np.ascontiguousarray(
            wq.reshape(P, NCHUNK * SLOTS * 2)).astype(np.float16)

        in_maps.append({"img8": shard, "widx": widx, "wab": wab})
    t0 = _t("host prep", t0)

    res = run_bass_kernel_spmd(nc, in_maps, core_ids=list(range(8)))
    t0 = _t("device run", t0)

    full = np.empty((N, H, W, C), dtype=np.float32)
    dq = f32(s / IMG_Q)
    for core in range(8):
        b, h = core // 2, core % 2
        # device wrote pixel-major: out.reshape(-1) is the half-frame in
        # row-major (row, col, ch) order already
        o = res.results[core]["out"].reshape(HALF, W, C)
        dst = full[b, h * HALF:(h + 1) * HALF]
        np.copyto(dst, o, casting="unsafe")
        dst -= f32(128.0)
        dst *= dq
    _t("postprocess", t0)
    return full


# revision 12
# speedup vs baseline: 1.4159x; 1.1811x over previous
"""Dense image warp (tfa.dense_image_warp semantics) on 8 Trainium2 NeuronCores.

The axon tunnel to the devices moves ~35 MB/s H2D / ~20 MB/s D2H, so the
kernel is wire-bound: the design minimizes bytes on the wire.

Sharding: pure data parallel, 8 shards = (batch 0..3) x (row-half 0..1);
each core warps 360 rows x 1280 cols x 16 ch of one frame.

Wire format (per core):
  - image shard as int8 (quantized to +-126.49/s, exact-int f32 math on
    device afterwards), ~7.9 MB instead of a 125 MB f32 quad table;
  - bilinear indices as compact int16 [16, n] (the 8x gpsimd-core
    replication the gather needs is done on device);
  - lerp weights as fp16 (upcast on device);
  - output as uint8 (result is a convex combination of int8 values, so it
    fits; bias +128 then RNE cast on device, dequantized on host).

Device algorithm per core:
  1. quad-table build: int8 image tiles -> SBUF, upcast to f32, then 4
     strided DMA writes per tile assemble the 256 B-per-position table
     [img[r,j], img[r,j+1], img[r+1,j], img[r+1,j+1]] in Internal DRAM
     (dma_gather requires elements and strides in 256 B units - pixel
     granularity indexing of the raw image is impossible);
  2. the four bilinear neighbours of every output pixel are fetched with
     ONE int16 dma_gather from that table (450 chunks of 1024 indices;
     chunk windows are band-rebased so indices fit int16);
  3. the two bilinear lerps run on the Vector engine with per-pixel fp16
     weights broadcast over the 16 channels; the +128 biased result is
     RNE-cast to uint8 and streamed out with large DMAs.

Host prepares (exact f32 math mirroring the reference): quantized image
shards, band-local indices in dma_gather's wrapped-16 layout, fp16 weights
in the gather's output layout; afterwards dequantizes the uint8 result.
"""

import os
import sys
import time

import numpy as np

import concourse.bass as bass
import concourse.mybir as mybir
from concourse import bacc
from concourse.tile import TileContext
from concourse.bass_utils import run_bass_kernel_spmd

# problem geometry (fixed per spec)
N, H, W, C = 4, 720, 1280, 16
HALF = H // 2                      # output rows per core
P = 128
K = 1024                           # indices per dma_gather (ring-safe)
SLOTS = K // P                     # 8
NCHUNK = (HALF * W) // K           # 450
G = 18                             # chunks per super-group
NSG = NCHUNK // G                  # 25
QROWS = H - 1                      # 719 quad rows
QCOLS = W - 1                      # 1279 quad cols
CW = 4 * C                         # 64 f32 = 256 B per quad position
IMG_Q = 126.49                     # int8 quant range; keeps hd+128 in [2,254]

_WRAP16 = np.arange(K // 16)[None, :] * 16 + np.arange(16)[:, None]
_REL = (np.arange(NCHUNK) * K) // W   # first output row (in-half) per chunk

_VERBOSE = bool(os.environ.get("KERNEL_VERBOSE"))


def _t(label, t0):
    if _VERBOSE:
        print(f"[kernel] {label}: {time.perf_counter() - t0:.3f}s",
              file=sys.stderr, flush=True)
    return time.perf_counter()


_PROGRAM_CACHE = {}


def _build_program(margin):
    key = margin
    if key in _PROGRAM_CACHE:
        return _PROGRAM_CACHE[key]
    win_len = 2 * margin + (K // W) + 4
    tq_rows = HALF + 2 * margin + (K // W) + 6
    rtiles = -(-(tq_rows + 1) // P)        # image-row tiles of 128
    rpad = rtiles * P

    nc = bacc.Bacc("TRN2", target_bir_lowering=False, debug=False, num_devices=8)
    img8 = nc.dram_tensor("img8", [rpad, W * C], mybir.dt.int8,
                          kind="ExternalInput")
    widx = nc.dram_tensor("widx", [16, NCHUNK * (K // 16)], mybir.dt.int16,
                          kind="ExternalInput")
    wab = nc.dram_tensor("wab", [P, NCHUNK * SLOTS * 2], mybir.dt.float16,
                         kind="ExternalInput")
    # pixel-major: row (ck, s), inner (p, c) -> linear pixel ck*1024+s*128+p,
    # so the host needs no permute, just dequantize.
    out = nc.dram_tensor("out", [NCHUNK * SLOTS, P * C], mybir.dt.int8,
                         kind="ExternalOutput")
    imgq = nc.dram_tensor("imgq", [tq_rows * QCOLS, CW], mybir.dt.float32,
                          kind="Internal")

    with TileContext(nc) as tc:
        # ---- phase 1: build the quad table in device DRAM ----
        qv = imgq.ap().rearrange("(r j) (s c) -> r j s c", j=QCOLS, s=4)
        ov = out.ap().rearrange("r (p c) -> p r c", p=P)
        with tc.tile_pool(name="bld", bufs=1) as bld:
            for t in range(rtiles):
                r0 = t * P
                t8 = bld.tile([P, W, C], mybir.dt.int8, tag="t8")
                nc.sync.dma_start(
                    out=t8[:].rearrange("p a b -> p (a b)"),
                    in_=img8[r0:r0 + P, :])
                tf = bld.tile([P, W, C], mybir.dt.float32, tag="tf")
                nc.scalar.copy(out=tf[:], in_=t8[:])

                # DMA dst dims [rows, QCOLS] merge into one descriptor dim
                # (row stride == QCOLS * 256 B), whose num_elem field is
                # 16-bit: keep rows-per-DMA * QCOLS <= 65535.
                RCH = 50
                q1 = min(r0 + P, tq_rows)          # slabs 0/1: rows r0..q1
                qa = max(r0 - 1, 0)                # slabs 2/3: rows qa..qb
                qb = min(r0 + P - 1, tq_rows)
                for ra in range(r0, q1, RCH):
                    rb = min(ra + RCH, q1)
                    nc.sync.dma_start(out=qv[ra:rb, :, 0],
                                      in_=tf[ra - r0:rb - r0, 0:QCOLS])
                    nc.sync.dma_start(out=qv[ra:rb, :, 1],
                                      in_=tf[ra - r0:rb - r0, 1:W])
                for ra in range(qa, qb, RCH):
                    rb = min(ra + RCH, qb)
                    nc.sync.dma_start(out=qv[ra:rb, :, 2],
                                      in_=tf[ra - r0 + 1:rb - r0 + 1, 0:QCOLS])
                    nc.sync.dma_start(out=qv[ra:rb, :, 3],
                                      in_=tf[ra - r0 + 1:rb - r0 + 1, 1:W])

        # ---- phase 2: gather + bilinear lerp ----
        with (
            tc.tile_pool(name="idx", bufs=2) as idx_pool,
            tc.tile_pool(name="w", bufs=2) as w_pool,
            tc.tile_pool(name="g", bufs=2) as g_pool,
            tc.tile_pool(name="t", bufs=2) as t_pool,
        ):
            iw = K // 16
            for sg in range(NSG):
                idx_t = idx_pool.tile([P, G * iw], mybir.dt.int16, tag="idx")
                for g in range(8):                 # replicate for 8 gpsimd cores
                    nc.sync.dma_start(
                        out=idx_t[g * 16:(g + 1) * 16, :],
                        in_=widx[:, sg * G * iw:(sg + 1) * G * iw])
                w16 = w_pool.tile([P, G * SLOTS, 2], mybir.dt.float16, tag="w16")
                nc.sync.dma_start(
                    out=w16[:].rearrange("p a b -> p (a b)"),
                    in_=wab[:, sg * G * SLOTS * 2:(sg + 1) * G * SLOTS * 2])
                w32 = w_pool.tile([P, G * SLOTS, 2], mybir.dt.float32, tag="w32")
                nc.scalar.copy(out=w32[:], in_=w16[:])

                g_t = g_pool.tile([P, G, SLOTS, CW], mybir.dt.float32, tag="g")
                for j in range(G):
                    off = _REL[sg * G + j] * QCOLS
                    nc.gpsimd.dma_gather(
                        out_ap=g_t[:, j],
                        in_ap=imgq[off:off + win_len * QCOLS, :],
                        idxs_ap=idx_t[:, j * iw:(j + 1) * iw],
                        num_idxs=K, num_idxs_reg=K, elem_size=CW,
                    )

                npx = G * SLOTS
                gv = g_t[:].rearrange("p a b c -> p (a b) c")     # [P, npx, 64]
                ax = w32[:, :, 0:1]
                ay = w32[:, :, 1:2]

                dif = t_pool.tile([P, npx, 32], mybir.dt.float32, tag="dif")
                nc.vector.tensor_tensor(out=dif[:], in0=gv[:, :, 32:64],
                                        in1=gv[:, :, 0:32],
                                        op=mybir.AluOpType.subtract)
                ay_b, dif_b = bass.broadcast_tensor_aps(ay, dif[:])
                nc.vector.tensor_tensor(out=dif[:], in0=dif_b, in1=ay_b,
                                        op=mybir.AluOpType.mult)
                nc.vector.tensor_tensor(out=dif[:], in0=dif[:],
                                        in1=gv[:, :, 0:32],
                                        op=mybir.AluOpType.add)
                hd = t_pool.tile([P, npx, 16], mybir.dt.float32, tag="hd")
                nc.vector.tensor_tensor(out=hd[:], in0=dif[:, :, 16:32],
                                        in1=dif[:, :, 0:16],
                                        op=mybir.AluOpType.subtract)
                ax_b, hd_b = bass.broadcast_tensor_aps(ax, hd[:])
                nc.vector.tensor_tensor(out=hd[:], in0=hd_b, in1=ax_b,
                                        op=mybir.AluOpType.mult)
                i8 = t_pool.tile([P, npx, C], mybir.dt.int8, tag="i8")
                nc.vector.tensor_tensor(out=i8[:], in0=hd[:],
                                        in1=dif[:, :, 0:16],
                                        op=mybir.AluOpType.add)
                nc.sync.dma_start(
                    out=ov[:, sg * G * SLOTS:(sg + 1) * G * SLOTS, :],
                    in_=i8[:])
    nc.compile()
    _PROGRAM_CACHE[key] = (nc, win_len, tq_rows, rpad)
    return _PROGRAM_CACHE[key]


_SCRATCH = {}


def kernel(image, flow):
    t0 = time.perf_counter()
    image = np.asarray(image, dtype=np.float32)
    flow = np.asarray(flow, dtype=np.float32)
    f32 = np.float32

    # max-abs without np.abs temporaries
    s = float(max(image.max(), -float(image.min())))
    margin = int(np.ceil(max(flow.max(), -float(flow.min())))) + 2
    nc, win_len, tq_rows, rpad = _build_program(margin)
    t0 = _t("program", t0)

    # int8 quantization; values are exact small ints in f32 on device.
    # Chunked + in-place into persistent scratch to avoid 236 MB temps.
    if not _SCRATCH:
        _SCRATCH["img8"] = np.empty((N, H, W * C), dtype=np.int8)
        _SCRATCH["tmp32"] = np.empty((H // 4, W * C), dtype=np.float32)
        _SCRATCH["shards"] = None
    img8_full = _SCRATCH["img8"]
    tmp32 = _SCRATCH["tmp32"]
    q = f32(IMG_Q / s)
    im2d = image.reshape(N * 4, H // 4, W * C)
    i82d = img8_full.reshape(N * 4, H // 4, W * C)
    for i in range(N * 4):
        np.multiply(im2d[i], q, out=tmp32)
        np.rint(tmp32, out=tmp32)
        np.copyto(i82d[i], tmp32, casting="unsafe")
    t0 = _t("quantize", t0)

    gi = np.arange(H, dtype=f32)[:, None]
    gj = np.arange(W, dtype=f32)[None, :]

    if _SCRATCH["shards"] is None or _SCRATCH["shards"][0].shape[0] != rpad:
        # pad rows stay zero across calls; only the valid span is rewritten
        _SCRATCH["shards"] = [np.zeros((rpad, W * C), dtype=np.int8)
                              for _ in range(8)]

    in_maps = []
    for core in range(8):
        b, h = core // 2, core % 2
        rows = slice(h * HALF, (h + 1) * HALF)

        # padded per-core int8 shard: shard row t <-> absolute row t + A
        A = h * HALF - margin
        shard = _SCRATCH["shards"][core]
        lo = max(0, A)
        hi = min(H, A + tq_rows + 1)
        shard[lo - A:hi - A] = img8_full[b, lo:hi]

        f0 = flow[b, rows, :, 0]
        f1 = flow[b, rows, :, 1]
        qy = gi[rows] - f0
        qx = gj - f1
        fy = np.clip(np.floor(qy), 0.0, f32(H - 2)).astype(f32)
        fx = np.clip(np.floor(qx), 0.0, f32(W - 2)).astype(f32)
        ayw = np.clip(qy - fy, 0.0, 1.0).astype(f32)
        axw = np.clip(qx - fx, 0.0, 1.0).astype(f32)
        iy = fy.astype(np.int32).reshape(NCHUNK, K)
        ix = fx.astype(np.int32).reshape(NCHUNK, K)

        lrow = iy - A - _REL[:, None]
        loc = lrow * QCOLS + ix
        assert loc.min() >= 0 and loc.max() < win_len * QCOLS, (
            f"core {core} index out of window")
        widx16 = loc.astype(np.int16)
        widx = np.ascontiguousarray(
            widx16[:, _WRAP16].transpose(1, 0, 2).reshape(16, NCHUNK * (K // 16)))

        wflat = np.stack([axw.reshape(-1), ayw.reshape(-1)], axis=-1)
        wq = wflat.reshape(NCHUNK, SLOTS, P, 2).transpose(2, 0, 1, 3)
        wab = np.ascontiguousarray(
            wq.reshape(P, NCHUNK * SLOTS * 2)).astype(np.float16)

        in_maps.append({"img8": shard, "widx": widx, "wab": wab})
    t0 = _t("host prep", t0)

    res = run_bass_kernel_spmd(nc, in_maps, core_ids=list(range(8)))
    t0 = _t("device run", t0)

    full = np.empty((N, H, W, C), dtype=np.float32)
    dq = f32(s / IMG_Q)
    for core in range(8):
        b, h = core // 2, core % 2
        # device wrote pixel-major: out.reshape(-1) is the half-frame in
        # row-major (row, col, ch) order already
        o = res.results[core]["out"].reshape(HALF, W, C)
        dst = full[b, h * HALF:(h + 1) * HALF]
        np.multiply(o, dq, out=dst)
    _t("postprocess", t0)
    return full


# revision 13
# speedup vs baseline: 1.6282x; 1.1500x over previous
"""Dense image warp (tfa.dense_image_warp semantics) on 8 Trainium2 NeuronCores.

The axon tunnel to the devices moves ~35 MB/s H2D / ~20 MB/s D2H, so the
kernel is wire-bound: the design minimizes bytes on the wire.

Sharding: pure data parallel, 8 shards = (batch 0..3) x (row-half 0..1);
each core warps 360 rows x 1280 cols x 16 ch of one frame.

Wire format (per core):
  - image shard as int8 (quantized to +-126.49/s, exact-int f32 math on
    device afterwards), ~7.9 MB instead of a 125 MB f32 quad table;
  - bilinear indices as compact int16 [16, n] (the 8x gpsimd-core
    replication the gather needs is done on device);
  - lerp weights as fp16 (upcast on device);
  - output as uint8 (result is a convex combination of int8 values, so it
    fits; bias +128 then RNE cast on device, dequantized on host).

Device algorithm per core:
  1. quad-table build: int8 image tiles -> SBUF, upcast to f32, then 4
     strided DMA writes per tile assemble the 256 B-per-position table
     [img[r,j], img[r,j+1], img[r+1,j], img[r+1,j+1]] in Internal DRAM
     (dma_gather requires elements and strides in 256 B units - pixel
     granularity indexing of the raw image is impossible);
  2. the four bilinear neighbours of every output pixel are fetched with
     ONE int16 dma_gather from that table (450 chunks of 1024 indices;
     chunk windows are band-rebased so indices fit int16);
  3. the two bilinear lerps run on the Vector engine with per-pixel fp16
     weights broadcast over the 16 channels; the +128 biased result is
     RNE-cast to uint8 and streamed out with large DMAs.

Host prepares (exact f32 math mirroring the reference): quantized image
shards, band-local indices in dma_gather's wrapped-16 layout, fp16 weights
in the gather's output layout; afterwards dequantizes the uint8 result.
"""

import os
import sys
import time

import numpy as np

import concourse.bass as bass
import concourse.mybir as mybir
from concourse import bacc
from concourse.tile import TileContext
from concourse.bass_utils import run_bass_kernel_spmd

# problem geometry (fixed per spec)
N, H, W, C = 4, 720, 1280, 16
HALF = H // 2                      # output rows per core
P = 128
K = 1024                           # indices per dma_gather (ring-safe)
SLOTS = K // P                     # 8
NCHUNK = (HALF * W) // K           # 450
G = 18                             # chunks per super-group
NSG = NCHUNK // G                  # 25
QROWS = H - 1                      # 719 quad rows
QCOLS = W - 1                      # 1279 quad cols
CW = 4 * C                         # 64 f32 = 256 B per quad position
IMG_Q = 126.49                     # int8 quant range; keeps hd+128 in [2,254]

_WRAP16 = np.arange(K // 16)[None, :] * 16 + np.arange(16)[:, None]
_REL = (np.arange(NCHUNK) * K) // W   # first output row (in-half) per chunk

_VERBOSE = bool(os.environ.get("KERNEL_VERBOSE"))


def _t(label, t0):
    if _VERBOSE:
        print(f"[kernel] {label}: {time.perf_counter() - t0:.3f}s",
              file=sys.stderr, flush=True)
    return time.perf_counter()


_PROGRAM_CACHE = {}


def _build_program(margin):
    key = margin
    if key in _PROGRAM_CACHE:
        return _PROGRAM_CACHE[key]
    win_len = 2 * margin + (K // W) + 4
    tq_rows = HALF + 2 * margin + (K // W) + 6
    rtiles = -(-(tq_rows + 1) // P)        # image-row tiles of 128
    rpad = rtiles * P

    nc = bacc.Bacc("TRN2", target_bir_lowering=False, debug=False, num_devices=8)
    img8 = nc.dram_tensor("img8", [rpad, W * C], mybir.dt.int8,
                          kind="ExternalInput")
    widx = nc.dram_tensor("widx", [16, NCHUNK * (K // 16)], mybir.dt.int16,
                          kind="ExternalInput")
    wab = nc.dram_tensor("wab", [P, NCHUNK * SLOTS * 2], mybir.dt.float16,
                         kind="ExternalInput")
    # pixel-major: row (ck, s), inner (p, c) -> linear pixel ck*1024+s*128+p,
    # so the host needs no permute, just dequantize.
    out = nc.dram_tensor("out", [NCHUNK * SLOTS, P * C], mybir.dt.int8,
                         kind="ExternalOutput")
    imgq = nc.dram_tensor("imgq", [tq_rows * QCOLS, CW], mybir.dt.float32,
                          kind="Internal")

    with TileContext(nc) as tc:
        # ---- phase 1: build the quad table in device DRAM ----
        qv = imgq.ap().rearrange("(r j) (s c) -> r j s c", j=QCOLS, s=4)
        ov = out.ap().rearrange("r (p c) -> p r c", p=P)
        with tc.tile_pool(name="bld", bufs=1) as bld:
            for t in range(rtiles):
                r0 = t * P
                t8 = bld.tile([P, W, C], mybir.dt.int8, tag="t8")
                nc.sync.dma_start(
                    out=t8[:].rearrange("p a b -> p (a b)"),
                    in_=img8[r0:r0 + P, :])
                tf = bld.tile([P, W, C], mybir.dt.float32, tag="tf")
                nc.scalar.copy(out=tf[:], in_=t8[:])

                # DMA dst dims [rows, QCOLS] merge into one descriptor dim
                # (row stride == QCOLS * 256 B), whose num_elem field is
                # 16-bit: keep rows-per-DMA * QCOLS <= 65535.
                RCH = 50
                q1 = min(r0 + P, tq_rows)          # slabs 0/1: rows r0..q1
                qa = max(r0 - 1, 0)                # slabs 2/3: rows qa..qb
                qb = min(r0 + P - 1, tq_rows)
                for ra in range(r0, q1, RCH):
                    rb = min(ra + RCH, q1)
                    nc.sync.dma_start(out=qv[ra:rb, :, 0],
                                      in_=tf[ra - r0:rb - r0, 0:QCOLS])
                    nc.sync.dma_start(out=qv[ra:rb, :, 1],
                                      in_=tf[ra - r0:rb - r0, 1:W])
                for ra in range(qa, qb, RCH):
                    rb = min(ra + RCH, qb)
                    nc.sync.dma_start(out=qv[ra:rb, :, 2],
                                      in_=tf[ra - r0 + 1:rb - r0 + 1, 0:QCOLS])
                    nc.sync.dma_start(out=qv[ra:rb, :, 3],
                                      in_=tf[ra - r0 + 1:rb - r0 + 1, 1:W])

        # ---- phase 2: gather + bilinear lerp ----
        with (
            tc.tile_pool(name="idx", bufs=2) as idx_pool,
            tc.tile_pool(name="w", bufs=2) as w_pool,
            tc.tile_pool(name="g", bufs=2) as g_pool,
            tc.tile_pool(name="t", bufs=2) as t_pool,
        ):
            iw = K // 16
            for sg in range(NSG):
                idx_t = idx_pool.tile([P, G * iw], mybir.dt.int16, tag="idx")
                for g in range(8):                 # replicate for 8 gpsimd cores
                    nc.sync.dma_start(
                        out=idx_t[g * 16:(g + 1) * 16, :],
                        in_=widx[:, sg * G * iw:(sg + 1) * G * iw])
                w16 = w_pool.tile([P, G * SLOTS, 2], mybir.dt.float16, tag="w16")
                nc.sync.dma_start(
                    out=w16[:].rearrange("p a b -> p (a b)"),
                    in_=wab[:, sg * G * SLOTS * 2:(sg + 1) * G * SLOTS * 2])
                w32 = w_pool.tile([P, G * SLOTS, 2], mybir.dt.float32, tag="w32")
                nc.scalar.copy(out=w32[:], in_=w16[:])

                g_t = g_pool.tile([P, G, SLOTS, CW], mybir.dt.float32, tag="g")
                for j in range(G):
                    off = _REL[sg * G + j] * QCOLS
                    nc.gpsimd.dma_gather(
                        out_ap=g_t[:, j],
                        in_ap=imgq[off:off + win_len * QCOLS, :],
                        idxs_ap=idx_t[:, j * iw:(j + 1) * iw],
                        num_idxs=K, num_idxs_reg=K, elem_size=CW,
                    )

                npx = G * SLOTS
                gv = g_t[:].rearrange("p a b c -> p (a b) c")     # [P, npx, 64]
                ax = w32[:, :, 0:1]
                ay = w32[:, :, 1:2]

                dif = t_pool.tile([P, npx, 32], mybir.dt.float32, tag="dif")
                nc.vector.tensor_tensor(out=dif[:], in0=gv[:, :, 32:64],
                                        in1=gv[:, :, 0:32],
                                        op=mybir.AluOpType.subtract)
                ay_b, dif_b = bass.broadcast_tensor_aps(ay, dif[:])
                nc.vector.tensor_tensor(out=dif[:], in0=dif_b, in1=ay_b,
                                        op=mybir.AluOpType.mult)
                nc.vector.tensor_tensor(out=dif[:], in0=dif[:],
                                        in1=gv[:, :, 0:32],
                                        op=mybir.AluOpType.add)
                hd = t_pool.tile([P, npx, 16], mybir.dt.float32, tag="hd")
                nc.vector.tensor_tensor(out=hd[:], in0=dif[:, :, 16:32],
                                        in1=dif[:, :, 0:16],
                                        op=mybir.AluOpType.subtract)
                ax_b, hd_b = bass.broadcast_tensor_aps(ax, hd[:])
                nc.vector.tensor_tensor(out=hd[:], in0=hd_b, in1=ax_b,
                                        op=mybir.AluOpType.mult)
                i8 = t_pool.tile([P, npx, C], mybir.dt.int8, tag="i8")
                nc.vector.tensor_tensor(out=i8[:], in0=hd[:],
                                        in1=dif[:, :, 0:16],
                                        op=mybir.AluOpType.add)
                nc.sync.dma_start(
                    out=ov[:, sg * G * SLOTS:(sg + 1) * G * SLOTS, :],
                    in_=i8[:])
    nc.compile()
    _PROGRAM_CACHE[key] = (nc, win_len, tq_rows, rpad)
    return _PROGRAM_CACHE[key]


_SCRATCH = {}


def kernel(image, flow):
    t0 = time.perf_counter()
    image = np.asarray(image, dtype=np.float32)
    flow = np.asarray(flow, dtype=np.float32)
    f32 = np.float32

    # max-abs without np.abs temporaries
    s = float(max(image.max(), -float(image.min())))
    margin = int(np.ceil(max(flow.max(), -float(flow.min())))) + 2
    nc, win_len, tq_rows, rpad = _build_program(margin)
    t0 = _t("program", t0)

    # int8 quantization; values are exact small ints in f32 on device.
    # Chunked + in-place into persistent scratch to avoid 236 MB temps.
    if not _SCRATCH:
        _SCRATCH["img8"] = np.empty((N, H, W * C), dtype=np.int8)
        _SCRATCH["tmp32"] = np.empty((H // 4, W * C), dtype=np.float32)
        _SCRATCH["shards"] = None
    img8_full = _SCRATCH["img8"]
    tmp32 = _SCRATCH["tmp32"]
    q = f32(IMG_Q / s)
    im2d = image.reshape(N * 4, H // 4, W * C)
    i82d = img8_full.reshape(N * 4, H // 4, W * C)
    for i in range(N * 4):
        np.multiply(im2d[i], q, out=tmp32)
        np.rint(tmp32, out=tmp32)
        np.copyto(i82d[i], tmp32, casting="unsafe")
    t0 = _t("quantize", t0)

    gi = np.arange(H, dtype=f32)[:, None]
    gj = np.arange(W, dtype=f32)[None, :]

    if _SCRATCH["shards"] is None or _SCRATCH["shards"][0].shape[0] != rpad:
        # pad rows stay zero across calls; only the valid span is rewritten
        _SCRATCH["shards"] = [np.zeros((rpad, W * C), dtype=np.int8)
                              for _ in range(8)]

    in_maps = []
    for core in range(8):
        b, h = core // 2, core % 2
        rows = slice(h * HALF, (h + 1) * HALF)

        # padded per-core int8 shard: shard row t <-> absolute row t + A
        A = h * HALF - margin
        shard = _SCRATCH["shards"][core]
        lo = max(0, A)
        hi = min(H, A + tq_rows + 1)
        shard[lo - A:hi - A] = img8_full[b, lo:hi]

        f0 = flow[b, rows, :, 0]
        f1 = flow[b, rows, :, 1]
        qy = gi[rows] - f0
        qx = gj - f1
        fy = np.clip(np.floor(qy), 0.0, f32(H - 2)).astype(f32)
        fx = np.clip(np.floor(qx), 0.0, f32(W - 2)).astype(f32)
        ayw = np.clip(qy - fy, 0.0, 1.0).astype(f32)
        axw = np.clip(qx - fx, 0.0, 1.0).astype(f32)
        iy = fy.astype(np.int32).reshape(NCHUNK, K)
        ix = fx.astype(np.int32).reshape(NCHUNK, K)

        lrow = iy - A - _REL[:, None]
        loc = lrow * QCOLS + ix
        assert loc.min() >= 0 and loc.max() < win_len * QCOLS, (
            f"core {core} index out of window")
        widx16 = loc.astype(np.int16)
        widx = np.ascontiguousarray(
            widx16[:, _WRAP16].transpose(1, 0, 2).reshape(16, NCHUNK * (K // 16)))

        wflat = np.stack([axw.reshape(-1), ayw.reshape(-1)], axis=-1)
        wq = wflat.reshape(NCHUNK, SLOTS, P, 2).transpose(2, 0, 1, 3)
        wab = np.ascontiguousarray(
            wq.reshape(P, NCHUNK * SLOTS * 2)).astype(np.float16)

        in_maps.append({"img8": shard, "widx": widx, "wab": wab})
    t0 = _t("host prep", t0)

    res = run_bass_kernel_spmd(nc, in_maps, core_ids=list(range(8)))
    t0 = _t("device run", t0)

    full = np.empty((N, H, W, C), dtype=np.float32)
    dq = f32(s / IMG_Q)
    cpu0 = time.process_time()
    for core in range(8):
        b, h = core // 2, core % 2
        # device wrote pixel-major: out.reshape(-1) is the half-frame in
        # row-major (row, col, ch) order already
        tc0 = time.perf_counter()
        o = res.results[core]["out"].reshape(HALF, W, C)
        dst = full[b, h * HALF:(h + 1) * HALF]
        np.multiply(o, dq, out=dst)
        if _VERBOSE:
            print(f"[kernel]   pp core{core}: {time.perf_counter() - tc0:.3f}s",
                  file=sys.stderr, flush=True)
    if _VERBOSE:
        print(f"[kernel]   pp cpu time: {time.process_time() - cpu0:.3f}s",
              file=sys.stderr, flush=True)
    _t("postprocess", t0)
    return full
